# revision 48
# baseline (speedup 1.0000x reference)
"""2-layer GCN (PyG GCNConv semantics) on 8 TRN2 NeuronCores — dst-major gather.

Strategy (vs the v1 indicator-matmul baseline, 3.41ms -> 2.97ms):
- Aggregation is destination-major: gather stream slot j*128+d serves
  dst partition d, so straight-gather blocks land [128 dst, 128 elems]
  (first 64 = message) and the segment sum is element-wise addition of
  blocks — a cross-ring pair tree of 3D-AP tensor_tensor adds (~8 DVE
  ops/tile, 64-elem contiguous runs). No indicator matrix load (116MB
  saved) and no per-block PE matmuls (7.7k matmul+ldweights removed).
- Pad slots are scattered across the 8 core blocks' zero rows: a single
  shared pad address serializes the SDMA drain (measured 16 -> 11 us
  per 4096-descriptor chunk after scattering). single_packet=True
  crashes the runtime; keep False.
- Per-(tile,residue) block counts are equalized across residues and
  chunks share tile-aligned boundaries across the 4 rings, so the four
  chunks covering a tile complete together and the in-order DVE
  consumer never head-blocks on a lagging ring.
- The int16 gather index addresses strides of 4 rows; each edge's slot
  must come from the residue view of its source's table position % 4.
  A host-side node *coloring* (greedy + 2 Gauss-Seidel passes, balanced
  per destination) picks each node's residue; dst tiles are then formed
  by clustering dsts with similar per-color in-degree profiles
  (per-class sorted dealing), which also satisfies the 32-per-residue
  position quota per tile. Per-(tile,residue) block counts nb[t][r] are
  uniform across cores/dsts; pad slots point at guaranteed-zero table
  rows (tile 97's reserved positions) so they add zero.
- x is pre-scaled by dinv[node] and pre-transposed on the host, so
  phase A is plain contiguous DMA + matmul, and h~ rows of pad
  positions are exactly zero. dinv of pad positions is 0 so layer-1
  epilogue (… + b1 → relu → ×dinv) also writes zero pad rows.
- log_softmax ln() via Newton iteration on ScalarE Exp (unchanged).
"""
import sys

sys.path.insert(0, "/opt/trn_rl_repo")

import numpy as np

import concourse.bass as bass
import concourse.bacc as bacc
import concourse.tile as tile
import concourse.mybir as mybir
from concourse import bass_utils
from concourse.masks import make_identity
from concourse.tile import add_dep_helper

N = 100000
F = 512
H = 64
CLS = 10
NC = 8
P = 128
NT = 98
NPAD = NT * P            # 12544 positions per core (incl. pads)
RG = [list(range(NC))]
NRES = 4
SMAX = NPAD * NC // NRES  # 25088 strides
TBL_ELEMS = NPAD * NC * H + 256
GB = 32                  # gather chunk size in 128-slot blocks
TG = 14                  # tiles per phase-A group (98 = 7*14)
SG = 14                  # tiles per softmax flush group
PAD_STRIDE = 3135        # rows 12540..12543 of core 0 block = zeros

BF16 = mybir.dt.bfloat16
F32 = mybir.dt.float32
I16 = mybir.dt.int16
NP_BF16 = mybir.dt.np(BF16)

_cache = {}


def _plan_chunks(L):
    """Group tiles into gather chunks of <= GB blocks, with IDENTICAL tile
    ranges across the 4 residue rings (L[t][r] is equal across r), so the
    four chunks covering a tile complete near-simultaneously and the
    in-order DVE consumer never head-blocks on a lagging ring.

    Returns per residue: list of chunks [(idx_col0, nidx, {t: block_off})],
    plus per (t, r): (chunk_id, block_off) and total idx-stream columns.
    """
    groups = []
    cur = []
    cur_b = 0
    for t in range(NT):
        b = L[t][0] // P
        if cur_b + b > GB and cur:
            groups.append(cur)
            cur = []
            cur_b = 0
        cur.append(t)
        cur_b += b
    if cur:
        groups.append(cur)

    chunks = [[] for _ in range(NRES)]
    where = {}
    col0 = 0
    for r in range(NRES):
        for ci, g in enumerate(groups):
            off = 0
            tmap = {}
            for t in g:
                tmap[t] = off
                where[(t, r)] = (ci, off)
                off += L[t][r] // P
            chunks[r].append((col0, off * P, tmap))
            col0 += off * P // 16
    return chunks, where, col0


def _build(Lkey):
    """Build + compile the SPMD graph for slot schedule L[t][r] (slots)."""
    L = [list(row) for row in Lkey]
    chunks, where, idx_cols = _plan_chunks(L)

    nc = bacc.Bacc("TRN2", target_bir_lowering=False, debug=False, num_devices=NC,
                   num_swdge_queues=4)

    xT_in = nc.dram_tensor("xT_in", [F, NPAD], BF16, kind="ExternalInput")
    w1_in = nc.dram_tensor("w1_in", [F, H], BF16, kind="ExternalInput")
    b1_in = nc.dram_tensor("b1_in", [P, H], F32, kind="ExternalInput")
    w2_in = nc.dram_tensor("w2_in", [H, CLS], BF16, kind="ExternalInput")
    b2_in = nc.dram_tensor("b2_in", [P, CLS], F32, kind="ExternalInput")
    dinv_in = nc.dram_tensor("dinv_in", [P, NT], F32, kind="ExternalInput")
    idx_in = nc.dram_tensor("idx_in", [P, idx_cols], I16, kind="ExternalInput")
    out = nc.dram_tensor("out", [NPAD, CLS], F32, kind="ExternalOutput")

    bounce1 = nc.dram_tensor("bounce1", [NPAD * H], BF16, kind="Internal")
    bounce2 = nc.dram_tensor("bounce2", [NPAD * H], BF16, kind="Internal")
    table1 = nc.dram_tensor("table1", [TBL_ELEMS], BF16, kind="Internal",
                            addr_space="Shared")
    table2 = nc.dram_tensor("table2", [TBL_ELEMS], BF16, kind="Internal",
                            addr_space="Shared")

    AF = mybir.ActivationFunctionType
    ALU = mybir.AluOpType

    def res_view(tbl, r):
        return tbl.ap()[r * H: r * H + SMAX * 4 * H].rearrange(
            "(s c) -> s c", c=4 * H)[:, 0:2 * H]

    with tile.TileContext(nc) as tc:
        with (
            tc.tile_pool(name="const", bufs=1) as constp,
            tc.tile_pool(name="xt", bufs=2) as xtp,
            tc.tile_pool(name="hpsum", bufs=3, space="PSUM") as hpsum,
            tc.tile_pool(name="hsb", bufs=4) as hsb,
            tc.tile_pool(name="idxp", bufs=5) as idxp,
            tc.tile_pool(name="ring0", bufs=4) as ring0,
            tc.tile_pool(name="ring1", bufs=4) as ring1,
            tc.tile_pool(name="ring2", bufs=4) as ring2,
            tc.tile_pool(name="smp", bufs=4) as smp,
            tc.tile_pool(name="trp", bufs=2) as trp,
            tc.tile_pool(name="ring3", bufs=4) as ring3,
            tc.tile_pool(name="psum2", bufs=2, space="PSUM") as psum2,
        ):
            rings = [ring0, ring1, ring2, ring3]
            # --- constants ---
            w1s = constp.tile([P, 4 * H], BF16)
            for k in range(4):
                nc.sync.dma_start(w1s[:, k * H:(k + 1) * H],
                                  w1_in.ap()[k * P:(k + 1) * P, :])
            b1s = constp.tile([P, H], F32)
            nc.sync.dma_start(b1s[:], b1_in.ap())
            w2s = constp.tile([H, CLS], BF16)
            nc.sync.dma_start(w2s[:], w2_in.ap())
            b2s = constp.tile([P, CLS], F32)
            nc.sync.dma_start(b2s[:], b2_in.ap())
            dinvs = constp.tile([P, NT], F32)
            nc.sync.dma_start(dinvs[:], dinv_in.ap())
            ident = constp.tile([P, P], BF16)
            make_identity(nc, ident[:])

            # --- phase A: h1~ = (dinv*x) @ W1 -> bounce1 (pad rows zero) ---
            b1v = bounce1.ap()[:].rearrange("(n f) -> n f", f=H)
            for g in range(NT // TG):
                xts = []
                for k in range(4):
                    xt = xtp.tile([P, TG * P], BF16, tag=f"xt{k}")
                    nc.sync.dma_start(
                        xt[:], xT_in.ap()[k * P:(k + 1) * P,
                                          g * TG * P:(g + 1) * TG * P])
                    xts.append(xt)
                for j in range(TG):
                    t = g * TG + j
                    ps = hpsum.tile([P, H], F32, tag="hps")
                    for k in range(4):
                        nc.tensor.matmul(ps[:], lhsT=xts[k][:, j * P:(j + 1) * P],
                                         rhs=w1s[:, k * H:(k + 1) * H],
                                         start=(k == 0), stop=(k == 3))
                    h1 = hsb.tile([P, H], BF16, tag="h1")
                    nc.vector.tensor_copy(out=h1[:], in_=ps[:])
                    nc.sync.dma_start(b1v[t * P:(t + 1) * P, :], h1[:])

            # AllGather in two 49-tile halves (half-major table layout):
            # the first half can fire while phase A still emits tiles 49..97.
            HE = (NPAD // 2) * H
            nc.gpsimd.collective_compute(
                "AllGather", ALU.bypass, replica_groups=RG,
                ins=[bounce1.ap()[0:HE].opt()],
                outs=[table1.ap()[0:HE * NC].opt()])
            nc.gpsimd.collective_compute(
                "AllGather", ALU.bypass, replica_groups=RG,
                ins=[bounce1.ap()[HE:2 * HE].opt()],
                outs=[table1.ap()[HE * NC:2 * HE * NC].opt()])

            # --- aggregation layers (dst-major gather + DVE reduce) ---
            MAXNB = max(L[t][r] // P for t in range(NT) for r in range(NRES))

            def agg_layer(table, post):
                views = [res_view(table, r) for r in range(NRES)]
                chunk_tiles = [dict() for _ in range(NRES)]
                chunk_insts = [dict() for _ in range(NRES)]

                def issue_chunk(r, ci):
                    col0, nidx, _tmap = chunks[r][ci]
                    cols = nidx // 16
                    it = idxp.tile([P, GB * 8], I16, tag=f"idx{r}")
                    # scalar (ACT) HWDGE ring: keeps idx prefetch out of the
                    # sync ring's FIFO, which carries epilogue output DMAs
                    # that wait on the DVE chain.
                    nc.scalar.dma_start(it[:, 0:cols],
                                        idx_in.ap()[:, col0:col0 + cols])
                    mt = rings[r].tile([P, GB * P], BF16, tag=f"msg{r}")
                    gi = nc.gpsimd.dma_gather(
                        mt[:, 0:nidx].rearrange("p (b f) -> p b f", f=P),
                        views[r], it[:, 0:cols], nidx, nidx, P,
                        elem_step=4 * H, single_packet=False, queue_num=r)
                    chunk_tiles[r][ci] = mt
                    chunk_insts[r][ci] = gi

                for r in range(NRES):
                    issue_chunk(r, 0)
                issued = [1] * NRES

                for t in range(NT):
                    # prefetch round-robin across rings per chunk level, so
                    # the four chunks of a tile group are generated adjacently
                    # and the in-order DVE consumer never waits on a ring
                    # whose chunk was queued 12 instructions later.
                    ci0, _ = where[(t, 0)]
                    while issued[0] <= ci0 + 3 and issued[0] < len(chunks[0]):
                        k = issued[0]
                        for r in range(NRES):
                            issue_chunk(r, k)
                            issued[r] += 1
                    # cross-ring pair-tree segment sum. Blocks are [128 dst,
                    # 128 elems] (first 64 = message). Round 1 sums ring
                    # pairs (bf16 -> f32 arena), round 2 merges the halves,
                    # then an in-place pair tree over nb chunks. 64-elem
                    # contiguous runs keep the DVE near line rate.
                    nbt = L[t][0] // P
                    ci, bo = where[(t, 0)]
                    mts = [chunk_tiles[r][ci] for r in range(NRES)]
                    gis = [chunk_insts[r][ci] for r in range(NRES)]

                    def hv(r):
                        return mts[r][:, bo * P:(bo + nbt) * P].rearrange(
                            "p (b f) -> p b f", f=P)[:, :, 0:H]

                    ar = trp.tile([P, 2 * MAXNB * H], F32, tag="tra")

                    def av(a, k):
                        return ar[:, a * H:(a + k) * H].rearrange(
                            "p (b f) -> p b f", f=H)

                    rA = nc.vector.tensor_tensor(out=av(0, nbt), in0=hv(0),
                                                 in1=hv(1), op=ALU.add)
                    add_dep_helper(rA.ins, gis[0].ins, reason="A waits g0")
                    add_dep_helper(rA.ins, gis[1].ins, reason="A waits g1")
                    rB = nc.vector.tensor_tensor(out=av(MAXNB, nbt), in0=hv(2),
                                                 in1=hv(3), op=ALU.add)
                    add_dep_helper(rB.ins, gis[2].ins, reason="B waits g2")
                    add_dep_helper(rB.ins, gis[3].ins, reason="B waits g3")
                    nc.vector.tensor_tensor(out=av(0, nbt), in0=av(0, nbt),
                                            in1=av(MAXNB, nbt), op=ALU.add)
                    k = nbt
                    pend = []
                    while k > 1:
                        k2 = k // 2
                        nc.vector.tensor_tensor(out=av(0, k2), in0=av(0, k2),
                                                in1=av(k2, k2), op=ALU.add)
                        if k % 2:
                            pend.append(k - 1)
                        k = k2
                    for pi in pend:
                        nc.vector.tensor_tensor(
                            out=ar[:, 0:H], in0=ar[:, 0:H],
                            in1=ar[:, pi * H:(pi + 1) * H], op=ALU.add)
                    post(t, ar[:, 0:H])

            # layer-1 epilogue: h2 = dinv * relu(dinv*agg + b1) -> bounce2
            b2v = bounce2.ap()[:].rearrange("(n f) -> n f", f=H)

            def post1(t, acc):
                y = smp.tile([P, H], F32, tag="y")
                nc.vector.tensor_scalar(out=y[:], in0=acc,
                                        scalar1=dinvs[:, t:t + 1], scalar2=None,
                                        op0=ALU.mult)
                y2 = smp.tile([P, H], F32, tag="y2")
                nc.vector.tensor_tensor(out=y2[:], in0=y[:], in1=b1s[:], op=ALU.add)
                h2 = hsb.tile([P, H], BF16, tag="h2")
                nc.vector.tensor_scalar(out=h2[:], in0=y2[:], scalar1=0.0,
                                        scalar2=dinvs[:, t:t + 1],
                                        op0=ALU.max, op1=ALU.mult)
                nc.sync.dma_start(b2v[t * P:(t + 1) * P, :], h2[:])

            agg_layer(table1, post1)

            nc.gpsimd.collective_compute(
                "AllGather", ALU.bypass, replica_groups=RG,
                ins=[bounce2.ap()[0:HE].opt()],
                outs=[table2.ap()[0:HE * NC].opt()])
            nc.gpsimd.collective_compute(
                "AllGather", ALU.bypass, replica_groups=RG,
                ins=[bounce2.ap()[HE:2 * HE].opt()],
                outs=[table2.ap()[HE * NC:2 * HE * NC].opt()])

            # layer-2 epilogue: out = log_softmax(dinv*agg @ W2 + b2)
            grp = {"buf": None, "t0": 0, "n": 0}

            def flush_group():
                ng = grp["n"]
                if ng == 0:
                    return
                yb = grp["buf"]
                t0 = grp["t0"]
                mx = smp.tile([P, SG], F32, tag="mx")
                nc.vector.tensor_reduce(
                    out=mx[:, 0:ng],
                    in_=yb[:, 0:ng * CLS].rearrange("p (j c) -> p j c", c=CLS),
                    axis=mybir.AxisListType.X, op=ALU.max)
                sh = smp.tile([P, SG * CLS], F32, tag="sh")
                nc.vector.tensor_tensor(
                    out=sh[:, 0:ng * CLS].rearrange("p (j c) -> p j c", c=CLS),
                    in0=yb[:, 0:ng * CLS].rearrange("p (j c) -> p j c", c=CLS),
                    in1=mx[:, 0:ng].unsqueeze(2).broadcast_to([P, ng, CLS]),
                    op=ALU.subtract)
                ex = smp.tile([P, SG * CLS], F32, tag="ex")
                nc.scalar.activation(out=ex[:, 0:ng * CLS], in_=sh[:, 0:ng * CLS],
                                     func=AF.Exp)
                sm = smp.tile([P, SG], F32, tag="sm")
                nc.vector.tensor_reduce(
                    out=sm[:, 0:ng],
                    in_=ex[:, 0:ng * CLS].rearrange("p (j c) -> p j c", c=CLS),
                    axis=mybir.AxisListType.X, op=ALU.add)
                # ls = ln(sm): Newton on f(y) = e^y - sm
                ls = smp.tile([P, SG], F32, tag="ls")
                nc.vector.tensor_scalar(out=ls[:, 0:ng], in0=sm[:, 0:ng],
                                        scalar1=0.2559, scalar2=-0.2559,
                                        op0=ALU.mult, op1=ALU.add)
                for _ in range(3):
                    en = smp.tile([P, SG], F32, tag="en")
                    nc.scalar.activation(out=en[:, 0:ng], in_=ls[:, 0:ng],
                                         func=AF.Exp, scale=-1.0)
                    pr = smp.tile([P, SG], F32, tag="pr")
                    nc.vector.tensor_tensor(out=pr[:, 0:ng], in0=en[:, 0:ng],
                                            in1=sm[:, 0:ng], op=ALU.mult)
                    ls2 = smp.tile([P, SG], F32, tag="ls")
                    nc.vector.tensor_tensor(out=ls2[:, 0:ng], in0=ls[:, 0:ng],
                                            in1=pr[:, 0:ng], op=ALU.add)
                    ls = ls2
                    nc.vector.tensor_scalar(out=ls[:, 0:ng], in0=ls[:, 0:ng],
                                            scalar1=1.0, scalar2=None,
                                            op0=ALU.subtract)
                res = smp.tile([P, SG * CLS], F32, tag="res")
                nc.vector.tensor_tensor(
                    out=res[:, 0:ng * CLS].rearrange("p (j c) -> p j c", c=CLS),
                    in0=sh[:, 0:ng * CLS].rearrange("p (j c) -> p j c", c=CLS),
                    in1=ls[:, 0:ng].unsqueeze(2).broadcast_to([P, ng, CLS]),
                    op=ALU.subtract)
                for j in range(ng):
                    t = t0 + j
                    nc.sync.dma_start(out.ap()[t * P:(t + 1) * P, :],
                                      res[:, j * CLS:(j + 1) * CLS])
                grp["buf"] = None
                grp["n"] = 0

            # post2 is software-pipelined: the psum->sbuf copy, W2 matmul and
            # bias-add for tile t run during tile t+1's processing, so the
            # in-order DVE queue never waits on the PE transpose it just fed.
            pend2 = {"t": None, "pt": None}

            def finish_pend2():
                if pend2["t"] is None:
                    return
                tp = pend2["t"]
                pt = pend2["pt"]
                aggT = smp.tile([H, P], BF16, tag="aggT")
                nc.vector.tensor_copy(out=aggT[:], in_=pt[:])
                po = psum2.tile([P, CLS], F32, tag="po")
                nc.tensor.matmul(po[:], lhsT=aggT[:], rhs=w2s[:], start=True,
                                 stop=True)
                if grp["buf"] is None:
                    grp["buf"] = smp.tile([P, SG * CLS], F32, tag="yb",
                                          name="yb")
                    grp["t0"] = tp
                j = grp["n"]
                nc.vector.tensor_tensor(
                    out=grp["buf"][:, j * CLS:(j + 1) * CLS], in0=po[:],
                    in1=b2s[:], op=ALU.add)
                grp["n"] += 1
                if grp["n"] == SG:
                    flush_group()
                pend2["t"] = None

            def post2(t, acc):
                aggb = smp.tile([P, H], BF16, tag="aggb")
                nc.vector.tensor_scalar(out=aggb[:], in0=acc,
                                        scalar1=dinvs[:, t:t + 1], scalar2=None,
                                        op0=ALU.mult)
                pt = psum2.tile([H, P], BF16, tag="pt")
                nc.tensor.transpose(out=pt[:], in_=aggb[:], identity=ident[:])
                finish_pend2()
                pend2["t"] = t
                pend2["pt"] = pt

            agg_layer(table2, post2)
            finish_pend2()
            flush_group()

    nc.compile()
    return nc


def _color_nodes(src_f, dst_f, deg):
    """Greedy + 2 Gauss-Seidel passes: per-dst balanced 4-coloring of nodes."""
    o = np.argsort(src_f, kind="stable")
    dst_s = dst_f[o]
    starts = np.zeros(N + 1, np.int64)
    np.cumsum(np.bincount(src_f, minlength=N), out=starts[1:])

    cnt = np.zeros((N, 4), np.float32)
    target = (deg / 4.0).astype(np.float32)
    color = np.full(N, -1, np.int8)
    class_sz = np.zeros(4, np.int64)
    CAP = 97 * 256 + 248  # per-class global capacity (positions per residue)
    BETA = 1.5
    rng = np.random.default_rng(1)
    proc = rng.permutation(N)
    for n in proc:
        ds = dst_s[starts[n]:starts[n + 1]]
        sc = np.exp(BETA * (cnt[ds] - target[ds][:, None])).sum(axis=0)
        sc[class_sz >= CAP] = np.inf
        r = int(np.argmin(sc))
        color[n] = r
        class_sz[r] += 1
        cnt[ds, r] += 1
    for _ in range(2):
        for n in proc:
            ds = dst_s[starts[n]:starts[n + 1]]
            r0 = color[n]
            cnt[ds, r0] -= 1
            sc = np.exp(BETA * (cnt[ds] - target[ds][:, None])).sum(axis=0)
            sc[class_sz >= CAP] = np.inf
            sc[r0] = np.exp(BETA * (cnt[ds, r0] - target[ds])).sum()
            r = int(np.argmin(sc))
            if r != r0:
                class_sz[r0] -= 1
                class_sz[r] += 1
            color[n] = r
            cnt[ds, r] += 1
    return color


def _prep(x, edge_index, W1, b1, W2, b2):
    """Host-side graph preprocessing."""
    x = np.asarray(x, dtype=np.float32)
    ei = np.asarray(edge_index, dtype=np.int64)
    W1 = np.asarray(W1, dtype=np.float32)
    b1 = np.asarray(b1, dtype=np.float32)
    W2 = np.asarray(W2, dtype=np.float32)
    b2 = np.asarray(b2, dtype=np.float32)

    nodes = np.arange(N, dtype=np.int64)
    src_f = np.concatenate([ei[0], nodes])
    dst_f = np.concatenate([ei[1], nodes])
    deg = np.bincount(dst_f, minlength=N)  # >= 1 (self-loops)
    dinv = (1.0 / np.sqrt(deg)).astype(np.float32)

    color = _color_nodes(src_f, dst_f, deg)
    cnt = np.zeros((N, 4), np.int32)
    np.add.at(cnt, (dst_f, color[src_f].astype(np.int64)), 1)
    key = cnt.max(axis=1)

    # per-class deal: sort class nodes by key desc; tiles 0..96 take 256
    # (32/core), tile 97 takes 248 (31/core); leftover positions j=31 of
    # tile 97 stay empty => rows 12540..12543 of each core block are zero.
    pos_of = np.full(N, -1, np.int64)
    nb = np.zeros((NT, NRES), np.int64)
    for r in range(4):
        cls = np.where(color == r)[0]
        cls = cls[np.argsort(-key[cls], kind="stable")]
        assert len(cls) >= 97 * 256, f"class {r} too small: {len(cls)}"
        assert len(cls) <= 97 * 256 + 248, f"class {r} too big: {len(cls)}"
        i = np.arange(len(cls), dtype=np.int64)
        t_n = np.minimum(i // 256, 97)
        jj = np.where(t_n < 97, i % 256, i - 97 * 256)
        cpc = np.where(t_n < 97, 32, 31)       # per-core count this tile
        c_n = jj // cpc
        j_n = jj % cpc
        pos_of[cls] = c_n * NPAD + t_n * P + 4 * j_n + r
    # nb[t, r] = max cnt[d, r] over all dsts in tile t (any core);
    # equalized across residues (near-free: the coloring balances them)
    # so all 4 rings share tile-aligned chunk boundaries.
    t_of = (pos_of % NPAD) // P
    for r in range(4):
        np.maximum.at(nb, (t_of, np.full(N, r)), cnt[:, r])
    nb = np.maximum(nb.max(axis=1, keepdims=True), 1).repeat(NRES, axis=1)
    assert (pos_of >= 0).all()

    core_of = pos_of // NPAD
    part_of = pos_of % P

    # --- edge streams: ring r holds, for each tile t, nb[t][r] blocks;
    # slot (block b, partition p) = b-th color-r source of dst at (c,t,p).
    r_e = color[src_f].astype(np.int64)
    dpos = pos_of[dst_f]
    c_e = dpos // NPAD
    t_e = (dpos % NPAD) // P
    p_e = dpos % P
    # table rows are half-major: [all cores' tiles 0..48; all cores' tiles
    # 49..97], so each AllGather half is contiguous and can overlap the
    # producer's second half. row % 4 == pos % 4 still holds (HALF % 4 == 0).
    HALF = NPAD // 2

    def table_row(pos):
        c = pos // NPAD
        j = pos % NPAD
        h = j // HALF
        return h * (HALF * NC) + c * HALF + (j - h * HALF)

    gid = dpos * 4 + r_e
    sidx = np.argsort(gid, kind="stable")
    gid_s = gid[sidx]
    q_s = (table_row(pos_of[src_f[sidx]]) // 4).astype(np.int16)
    ngid = NC * NPAD * 4
    gstarts = np.zeros(ngid + 1, np.int64)
    np.cumsum(np.bincount(gid_s, minlength=ngid), out=gstarts[1:])
    j_e = np.arange(len(gid_s), dtype=np.int64) - gstarts[gid_s]
    ce = c_e[sidx]
    te = t_e[sidx]
    pe = p_e[sidx]
    re = r_e[sidx]
    assert (j_e < nb[te, re]).all()

    tile_base = np.zeros((NT, NRES), np.int64)  # block offset within ring
    for r in range(4):
        tile_base[1:, r] = np.cumsum(nb[:-1, r])
    ring_sz = nb.sum(axis=0) * P                # slots per ring
    ring_base = np.zeros(NRES + 1, np.int64)
    np.cumsum(ring_sz, out=ring_base[1:])
    SL_tot = int(ring_base[-1])

    # dst-round-robin within each (tile, residue) stream: slot = j*128 + d,
    # so straight-gather blocks land [dst partition, 128 elems] and the
    # segment sum is element-wise addition of blocks.
    gpos = ring_base[re] + (tile_base[te, re] + j_e) * P + pe
    # pad slots point at the zero rows (positions 12540..12543 of each core
    # block); scatter them across the 8 core blocks so no single HBM region
    # becomes a hot bank. In half-major rows: 12540 -> HALF*NC + c*HALF +
    # (12540 - HALF).
    pad_strides = ((HALF * NC + np.arange(NC, dtype=np.int64) * HALF
                    + (12540 - HALF)) // 4).astype(np.int16)
    rng_pad = np.random.default_rng(7)
    idx_all = pad_strides[rng_pad.integers(0, NC, (NC, SL_tot))]
    idx_all[ce, gpos] = q_s

    # wrap idx streams: position g -> [g%16, g//16], replicated 8x
    idx_wrap = idx_all.reshape(NC, SL_tot // 16, 16).transpose(0, 2, 1)
    idx_wrap = np.tile(idx_wrap, (1, 8, 1))     # [NC, 128, SL/16]

    # dinv per position; pads (empty positions) get 0 so layer-1 epilogue
    # writes zero rows for them (b1-add would otherwise contaminate).
    dinv_pos = np.zeros((NC, NPAD), np.float32)
    node_of_pos = np.full((NC, NPAD), -1, np.int64)
    node_of_pos[core_of, pos_of % NPAD] = nodes
    filled = node_of_pos >= 0
    dinv_pos[filled] = dinv[node_of_pos[filled]]
    dinvT = np.ascontiguousarray(
        dinv_pos.reshape(NC, NT, P).transpose(0, 2, 1))  # [NC, P, NT]

    # x pre-scaled by dinv, transposed; pad columns zero -> zero h~ rows.
    x_pre = (dinv[:, None] * x).astype(NP_BF16)
    W1_bf = np.ascontiguousarray(W1.astype(NP_BF16))
    W2_bf = np.ascontiguousarray(W2.astype(NP_BF16))
    b1_bc = np.ascontiguousarray(
        np.broadcast_to(b1[None, :], (P, H)).astype(np.float32))
    b2_bc = np.ascontiguousarray(
        np.broadcast_to(b2[None, :], (P, CLS)).astype(np.float32))

    in_maps = []
    for cc in range(NC):
        xT = np.zeros((F, NPAD), dtype=NP_BF16)
        sel = node_of_pos[cc]
        m = sel >= 0
        xT[:, m] = x_pre[sel[m]].T
        in_maps.append({
            "xT_in": np.ascontiguousarray(xT),
            "w1_in": W1_bf,
            "b1_in": b1_bc,
            "w2_in": W2_bf,
            "b2_in": b2_bc,
            "dinv_in": np.ascontiguousarray(dinvT[cc]),
            "idx_in": np.ascontiguousarray(idx_wrap[cc]),
        })
    Lkey = tuple(tuple(int(v) * P for v in row) for row in nb)
    return Lkey, in_maps, node_of_pos


def _get_nc(Lkey):
    if Lkey not in _cache:
        _cache[Lkey] = _build(Lkey)
    return _cache[Lkey]


def run(x, edge_index, W1, b1, W2, b2, trace=False):
    Lkey, in_maps, node_of_pos = _prep(x, edge_index, W1, b1, W2, b2)
    nc = _get_nc(Lkey)
    res = bass_utils.run_bass_kernel_spmd(
        nc, in_maps, core_ids=list(range(NC)), trace=trace)
    out_full = np.empty((N, CLS), np.float32)
    for c in range(NC):
        oc = res.results[c]["out"]
        sel = node_of_pos[c]
        m = sel >= 0
        out_full[sel[m]] = oc[m.nonzero()[0]]
    return out_full, res


def kernel(x, edge_index, W1, b1, W2, b2):
    out_full, _ = run(x, edge_index, W1, b1, W2, b2)
    return out_full
